# revision 66
# baseline (speedup 1.0000x reference)
"""Multi-head attention Trainium2 kernel (B=2, S=4096, D=512, H=8).

Sharding: 8 cores = (batch b) x (query half qh: 2048 rows) x
(head group hg: 4 heads = 256 model dims).  Each core:
  - Q projection for its 2048 queries x its 256 dims
  - K/V projections for the full 4096 keys x its 256 dims
  - scores/softmax/PV for its 4 heads x its 2048 queries
  - partial output projection [2048, 512] through its 256 Wo rows
The host sums the two head-groups' partial outputs per (b, qh).

Schedule (v3): 8 segments = (query-quarter qq 0..3) x (head-pair hp 0..1),
16 step-PAIRS each.  Per pair the four score matmuls issue E,O,E,O
(row groups 0:64/64:128 alternate; the PE shares one 2.4GHz column bus,
so batching minimizes full-array stream switches: 2 per pair instead of
6), each step's E|O lands in one merged [128,1024] PSUM tile covered by
a single 1024-wide ScalarE exp, VectorE multiplies the halves by the
shared bf16 mask tile, and the four lagged PV matmuls (va stationary
65-wide: 64 dims + a 2^-64-scaled ones column accumulating the softmax
denominator; contract 128 so PV can never row-tile with anything)
run as one batch.  PSUM: 4 banks scores (bufs=2) + 2 banks PV
accumulators + 2 banks shared projection tag "px".
All projections (K/Q/V chunks, output projection) are interleaved into
the attention steps just ahead of use, so the exp stream starts ~15us
in and the tail is only the last quarter's norm drain.
Softmax normalization (no DRAM bounces): 1/den = exp(-ln(den*2^-64)
- 64*ln2), Ln and Exp sharing one ScalarE activation table; the den
row pre-scale keeps it inside the Ln table's valid window
[3.7e-20, 2e19] for the full f32 range; the [1,512] lnden row is
broadcast to 64 partitions by a tiny f32r PE matmul and the -64*ln2
compensation rides the Exp activation's bias input.
Output projection contracts 128 rows per matmul: at2[hp] stacks each
head-pair's 64+64 dims (odd heads reach partitions 64:128 via an
SBUF->SBUF DMA, the only partition-crossing engine).
DMA: critical prefix (biases, wk, xk-g0) leads the SP ring ahead of
the per-step mask stream; wq/xq lead the GpSimd SWDGE ring followed by
all bulk inputs - keeping long trigger chains off the compute engines
(a trigger chain blocks its engine until the ring drains, which cost
the old layout ~50us of startup).  Output writes go via the Activation
ring, idle after startup.
Known variance: a chip-wide clock governor moves all engines between
~0.65x and ~1.0x in windows; identical code measures ~365-440us.
"""

import numpy as np
import ml_dtypes

import bass_rust
import concourse.bass as bass
import concourse.mybir as mybir
from concourse.bass_utils import run_bass_kernel_spmd
from concourse.tile import TileContext

f32 = mybir.dt.float32
f32r = mybir.dt.float32r
bf16 = mybir.dt.bfloat16
f16 = mybir.dt.float16
AF = mybir.ActivationFunctionType
MULT = mybir.AluOpType.mult

B, S, D, H, HD = 2, 4096, 512, 8, 64
NC = 8
QC = 2048          # query rows per core
DG = 256           # model dims per core (4 heads)
NH = 4             # heads per core
NKB = S // 128     # 32 k-blocks
NDB = DG // 128    # 2 d-blocks (head pairs) per core
NSB = QC // 128    # 16 out s-blocks
NQQ = 4            # query quarters (512 each)

_waitfix = [0]


def _legalize_waits(nc):
    """This walrus build accepts at most one sem-wait per instruction.
    Hoist extra waits onto same-engine NOPs inserted just before."""
    for fn in nc.m.functions:
        for bb in fn.blocks:
            out, changed = [], False
            for inst in bb.instructions:
                si = inst.sync_info
                if si is not None and len(si.on_wait) > 1:
                    waits = list(si.on_wait)
                    for w in waits[:-1]:
                        _waitfix[0] += 1
                        nop = mybir.InstNoOp(
                            name=f"I-waitfix-{_waitfix[0]}", ins=[], outs=[])
                        nop.engine = inst.engine
                        nop.sync_info = bass_rust.SyncInfo(on_wait=[w], on_update=[])
                        out.append(nop)
                    inst.sync_info = bass_rust.SyncInfo(
                        on_wait=[waits[-1]], on_update=list(si.on_update))
                    changed = True
                out.append(inst)
            if changed:
                bb.instructions = out


def _build_program(with_bias=False):
    nc = bass.Bass(target_bir_lowering=False, debug=False)

    xqT = nc.dram_tensor("xqT", [D, QC], f16, kind="ExternalInput")
    xkT = nc.dram_tensor("xkT", [D, S], f16, kind="ExternalInput")
    xvT = nc.dram_tensor("xvT", [D, S], f16, kind="ExternalInput")
    masktd = nc.dram_tensor("masktd", [S, QC], bf16, kind="ExternalInput")
    wqT = nc.dram_tensor("wqT", [D, DG], f16, kind="ExternalInput")
    wkT = nc.dram_tensor("wkT", [D, DG], f16, kind="ExternalInput")
    wvT = nc.dram_tensor("wvT", [D, DG], f16, kind="ExternalInput")
    woT = nc.dram_tensor("woT", [DG, D], f16, kind="ExternalInput")
    bq_d = nc.dram_tensor("bq_d", [128, NDB], f32, kind="ExternalInput")
    bk_d = nc.dram_tensor("bk_d", [128, NDB], f32, kind="ExternalInput")
    bv_d = nc.dram_tensor("bv_d", [1, DG], f16, kind="ExternalInput")
    bo_d = nc.dram_tensor("bo_d", [1, D], f16, kind="ExternalInput")
    outp = nc.dram_tensor("out", [QC, D], f32, kind="ExternalOutput")

    with TileContext(nc) as tc:
        with tc.tile_pool(name="cpool", bufs=1) as cpool, \
             tc.tile_pool(name="bpool", bufs=1) as bpool, \
             tc.tile_pool(name="psB", bufs=1, space="PSUM") as psB:
            # ---- constants ----
            ones_f = cpool.tile([1, 128], f32, tag="ones_f")
            nc.vector.memset(ones_f[:, :], 1.0)
            ones_r = cpool.tile([1, 128], f32r, tag="ones_r")
            nc.vector.tensor_copy(ones_r[:, :], ones_f[:, :])
            negones_f = cpool.tile([65, 128], f32, tag="negones_f")
            nc.vector.memset(negones_f[64:65, :], -1.0)
            negones_r = cpool.tile([65, 128], f32r, tag="negones_r")
            nc.vector.tensor_copy(negones_r[64:65, :], negones_f[64:65, :])
            # den is accumulated pre-scaled by 2^-64 (the va ones column) so
            # it stays inside the Ln table's valid window [3.7e-20, 2.07e19]
            # for the whole f32 range of den; the -64*ln2 term is added back
            # before the exp so invd = exp(-ln(den*2^-64) - 64*ln2) = 1/den.
            ones_h = cpool.tile([1, 128], f16, tag="ones_h")
            nc.vector.tensor_copy(ones_h[:, :], ones_f[:, :])
            nrm_b = cpool.tile([64, 1], f32, tag="nrm_b")
            nc.vector.memset(nrm_b[:, :], -64.0 * float(np.log(2.0)))
            expwarm = cpool.tile([1, 128], f32, tag="expwarm")
            nc.scalar.activation(expwarm[:, :], ones_f[:, :], AF.Exp)

            # ---- critical input prefix on the SP(sync) ring (fast HWDGE,
            # ahead of the mask stream); everything else via GpSimd SWDGE
            # so no compute engine ever blocks issuing DMA triggers ----
            bq_t = cpool.tile([128, NDB], f32, tag="bq")
            bk_t = cpool.tile([128, NDB], f32, tag="bk")
            nc.sync.dma_start(out=bq_t[:, :], in_=bq_d[:, :])
            nc.sync.dma_start(out=bk_t[:, :], in_=bk_d[:, :])
            wk_c, wq_c, wv_c = [], [], []
            for kc in range(4):
                t = cpool.tile([128, DG], f16, tag=f"wk{kc}", name=f"wk{kc}")
                nc.sync.dma_start(out=t[:, :], in_=wkT[kc * 128:(kc + 1) * 128, :])
                wk_c.append(t)
            xk_c = []
            for kc in range(4):
                xk_c.append(cpool.tile([128, S], f16, tag=f"xk{kc}", name=f"xk{kc}"))
            # xk-g0 split into 512-col halves, all kc's half-0 first, so
            # the first k-half emission starts as soon as 512KB lands
            for hf in range(2):
                for kc in range(4):
                    nc.sync.dma_start(
                        out=xk_c[kc][:, hf * 512:(hf + 1) * 512],
                        in_=xkT[kc * 128:(kc + 1) * 128,
                                hf * 512:(hf + 1) * 512])
            # wq + xq-q0 go at the FRONT of the GpSimd ring so they stream
            # in parallel with the sync-ring prefix
            for kc in range(4):
                t = cpool.tile([128, DG], f16, tag=f"wq{kc}", name=f"wq{kc}")
                nc.gpsimd.dma_start(out=t[:, :], in_=wqT[kc * 128:(kc + 1) * 128, :])
                wq_c.append(t)
            xq_c = []
            for kc in range(4):
                xq_c.append(cpool.tile([128, QC], f16, tag=f"xq{kc}", name=f"xq{kc}"))
            for kc in range(4):
                nc.gpsimd.dma_start(out=xq_c[kc][:, 0:512],
                                    in_=xqT[kc * 128:(kc + 1) * 128, 0:512])
            bv_t = cpool.tile([1, DG], f16, tag="bv")
            bo_t = cpool.tile([1, D], f16, tag="bo")

            # ---- bulk inputs on GpSimd SWDGE, in dependency-priority order
            xv_c = []
            for kc in range(4):
                xv_c.append(cpool.tile([128, S], f16, tag=f"xv{kc}", name=f"xvc{kc}"))
            for kc in range(4):
                nc.gpsimd.dma_start(out=xv_c[kc][:, 0:1024],
                                    in_=xvT[kc * 128:(kc + 1) * 128, 0:1024])
            for kc in range(4):
                t = cpool.tile([128, DG], f16, tag=f"wv{kc}", name=f"wv{kc}")
                nc.gpsimd.dma_start(out=t[:, :], in_=wvT[kc * 128:(kc + 1) * 128, :])
                wv_c.append(t)
            nc.gpsimd.dma_start(out=bv_t[:, :], in_=bv_d[:, :])
            for kc in range(4):
                nc.gpsimd.dma_start(
                    out=xk_c[kc][:, 1024:2048], in_=xkT[kc * 128:(kc + 1) * 128, 1024:2048])
            for kc in range(4):
                nc.gpsimd.dma_start(out=xv_c[kc][:, 1024:2048],
                                    in_=xvT[kc * 128:(kc + 1) * 128, 1024:2048])
            for kc in range(4):
                nc.gpsimd.dma_start(
                    out=xk_c[kc][:, 2048:3072], in_=xkT[kc * 128:(kc + 1) * 128, 2048:3072])
            for kc in range(4):
                nc.gpsimd.dma_start(
                    out=xk_c[kc][:, 3072:4096], in_=xkT[kc * 128:(kc + 1) * 128, 3072:4096])
            for kc in range(4):
                nc.gpsimd.dma_start(out=xv_c[kc][:, 2048:3072],
                                    in_=xvT[kc * 128:(kc + 1) * 128, 2048:3072])
            wo2 = []
            for hp in range(NDB):
                t = cpool.tile([128, D], f16, tag=f"wo2{hp}", name=f"wo2{hp}")
                nc.gpsimd.dma_start(out=t[:, :], in_=woT[hp * 128:(hp + 1) * 128, :])
                wo2.append(t)
            nc.gpsimd.dma_start(out=bo_t[:, :], in_=bo_d[:, :])
            for kc in range(4):
                nc.gpsimd.dma_start(out=xv_c[kc][:, 3072:4096],
                                    in_=xvT[kc * 128:(kc + 1) * 128, 3072:4096])
            for qq in range(1, 4):
                for kc in range(4):
                    nc.gpsimd.dma_start(
                        out=xq_c[kc][:, qq * 512:(qq + 1) * 512],
                        in_=xqT[kc * 128:(kc + 1) * 128, qq * 512:(qq + 1) * 512])

            # ---- PE warmup (ramp cover while DMA streams) ----
            ones_w = cpool.tile([128, 512], f16, tag="ones_w")
            nc.vector.memset(ones_w[:, :], 1.0)
            wps = psB.tile([128, 1024], f32, tag="sc", bufs=2, name="wps")
            for r in range(22):
                nc.tensor.matmul(wps[:, 0:512], ones_w[:, 0:128],
                                 ones_w[:, :], start=(r == 0), stop=(r == 21))
            wout = cpool.tile([1, 512], f32, tag="wout")
            nc.vector.tensor_copy(wout[:, :], wps[0:1, 0:512])

            # ---- persistent activations ----
            qt = [cpool.tile([128, QC], f16, tag=f"qt{db}", name=f"qt{db}")
                  for db in range(NDB)]
            kt = [cpool.tile([128, S], f16, tag=f"kt{db}", name=f"kt{db}")
                  for db in range(NDB)]
            va = [cpool.tile([128, NH * 65], bf16, tag=f"va{sb}", name=f"va{sb}")
                  for sb in range(NKB)]
            # at2[hp]: head 2hp dims at partitions 0:64, head 2hp+1 at 64:128
            at2 = [cpool.tile([128, QC], f16, tag=f"at2{hp}", name=f"at2{hp}")
                   for hp in range(NDB)]

            # ---------- projection emitters (all via 1-bank "px" tag) ----------
            def emit_k_half(db, g, hf):
                """kt[db][:, g*1024+hf*512 : ...+512] from xk chunk."""
                ps = psB.tile([128, 512], f32, tag="px", bufs=2,
                              name=f"kps{db}{g}{hf}")
                cs = slice(g * 1024 + hf * 512, g * 1024 + (hf + 1) * 512)
                for kc in range(4):
                    nc.tensor.matmul(ps[:, :],
                                     wk_c[kc][:, db * 128:(db + 1) * 128],
                                     xk_c[kc][:, cs],
                                     start=(kc == 0), stop=(kc == 3))
                if with_bias:
                    nc.scalar.activation(kt[db][:, cs], ps[:, :],
                                         AF.Identity, bias=bk_t[:, db:db + 1])
                else:
                    with nc.allow_low_precision(reason="f16 kt"):
                        nc.vector.tensor_copy(kt[db][:, cs], ps[:, :])

            def emit_q_quarter(db, qq):
                """qt[db][:, qq*512:(qq+1)*512]."""
                ps = psB.tile([128, 512], f32, tag="px", bufs=2,
                              name=f"qps{db}{qq}")
                cs = slice(qq * 512, (qq + 1) * 512)
                for kc in range(4):
                    nc.tensor.matmul(ps[:, :],
                                     wq_c[kc][:, db * 128:(db + 1) * 128],
                                     xq_c[kc][:, cs],
                                     start=(kc == 0), stop=(kc == 3))
                if with_bias:
                    nc.scalar.activation(qt[db][:, cs], ps[:, :],
                                         AF.Identity, bias=bq_t[:, db:db + 1])
                else:
                    with nc.allow_low_precision(reason="f16 qt"):
                        nc.vector.tensor_copy(qt[db][:, cs], ps[:, :])

            def emit_v_chunk(sb):
                ps = psB.tile([128, 512], f32, tag="px", bufs=2,
                              name=f"vps{sb}")[:, 0:DG]
                for kc in range(4):
                    nc.tensor.matmul(ps[:, :],
                                     xv_c[kc][:, sb * 128:(sb + 1) * 128],
                                     wv_c[kc][:, :],
                                     start=(kc == 0),
                                     stop=(kc == 3 and not with_bias))
                if with_bias:
                    nc.tensor.matmul(ps[:, :], ones_h[0:1, :], bv_t[0:1, :],
                                     start=False, stop=True)
                dst = va[sb][:, :].rearrange("p (h c) -> p h c", c=65)
                src = ps[:, :].rearrange("p (h c) -> p h c", c=64)
                with nc.allow_low_precision(reason="bf16 va"):
                    nc.vector.tensor_copy(dst[:, :, 0:64], src[:, :, :])
                nc.vector.memset(dst[:, :, 64:65], 2.0 ** -64)

            def emit_out_proj(sb):
                po = psB.tile([128, 512], f32, tag="px", bufs=2,
                              name=f"po{sb}")
                for hp in range(NDB):
                    nc.tensor.matmul(
                        po[:, :],
                        at2[hp][:, sb * 128:(sb + 1) * 128],
                        wo2[hp][:, :],
                        start=(hp == 0), stop=(hp == NDB - 1 and not with_bias))
                if with_bias:
                    nc.tensor.matmul(po[:, :], ones_h[0:1, :], bo_t[0:1, :],
                                     start=False, stop=True)
                osb = bpool.tile([128, D], f32, tag="osb", bufs=2,
                                 name=f"osb{sb}")
                nc.vector.tensor_copy(osb[:, :], po[:, :])
                nc.scalar.dma_start(out=outp[sb * 128:(sb + 1) * 128, :],
                                    in_=osb[:, :])

            # ---------- normalization: 1/den = exp(-ln(den)), all on
            # ScalarE (Ln and Exp share one activation table) ----------
            stgs, lndens, stg2s = {}, {}, {}

            def emit_norm_stg(h, qq, pv):
                stg = bpool.tile([65, 512], f32, tag="stg", bufs=3,
                                 name=f"stg{h}{qq}")
                nc.vector.tensor_copy(stg[:, :], pv[:, :])
                stgs[(h, qq)] = stg

            def emit_norm_ln(h, qq):
                stg = stgs[(h, qq)]
                lnden = bpool.tile([65, 512], f32r, tag="lnden", bufs=3,
                                   name=f"ln{h}{qq}")
                nc.scalar.activation(lnden[64:65, :], stg[64:65, :], AF.Ln)
                lndens[(h, qq)] = lnden

            def emit_norm_rest(h, qq):
                stg = stgs.pop((h, qq))
                lnden = lndens.pop((h, qq))
                qs = slice(qq * 512, (qq + 1) * 512)
                bc = psB.tile([128, 512], f32, tag="px", bufs=2,
                              name=f"bc{h}{qq}")
                nc.tensor.matmul(bc[0:64, :], negones_r[64:65, 0:64],
                                 lnden[64:65, :], start=True, stop=True)
                # invd = exp(-lnden - 64*ln2) = 1/den (the 2^-64 den
                # pre-scale is undone by the activation bias)
                invd = bpool.tile([64, 512], f32, tag="invd", bufs=2,
                                  name=f"invd{h}{qq}")
                nc.scalar.activation(invd[:, :], bc[0:64, :], AF.Exp,
                                     bias=nrm_b[:, 0:1])
                with nc.allow_low_precision(reason="f16 at"):
                    if h % 2 == 0:
                        nc.vector.tensor_tensor(
                            at2[h // 2][0:64, qs],
                            stg[0:64, :], invd[:, :], op=MULT)
                    else:
                        # odd heads land at partitions 64:128 of at2; only
                        # DMA crosses partitions, so stage through SBUF
                        ato = bpool.tile([64, 512], f16, tag="ato", bufs=2,
                                         name=f"ato{h}{qq}")
                        nc.vector.tensor_tensor(
                            ato[:, :], stg[0:64, :], invd[:, :], op=MULT)
                        nc.gpsimd.dma_start(
                            out=at2[h // 2][64:128, qs], in_=ato[:, :])

            # ---------- attention ----------
            LAG = 3            # probs-entries PV trails behind scores
            NORM_DELAY = 4     # in step-PAIRS (run_pend_norm per pair)
            pvt = {}
            pend_pv = []       # (h, qq, kb, probs, half)
            pend_norm = []     # [delay, h, qq, stage]

            def emit_pv(h, qq, kb, probs, hf):
                nc.tensor.matmul(
                    pvt[(h, qq)][:, :],
                    va[kb][:, h * 65:(h + 1) * 65],
                    probs[:, hf * 512:(hf + 1) * 512],
                    start=(kb == 0), stop=(kb == NKB - 1))

            def run_pend_norm():
                for ent in pend_norm:
                    ent[0] -= 1
                    if ent[0] == NORM_DELAY - 1 and ent[3] == 0:
                        emit_norm_stg(ent[1], ent[2], pvt.pop((ent[1], ent[2])))
                        ent[3] = 1
                    elif ent[0] == NORM_DELAY - 3 and ent[3] == 1:
                        emit_norm_ln(ent[1], ent[2])
                        ent[3] = 2
                while pend_norm and pend_norm[0][0] <= 0:
                    e = pend_norm.pop(0)
                    if e[3] < 1:
                        emit_norm_stg(e[1], e[2], pvt.pop((e[1], e[2])))
                    if e[3] < 2:
                        emit_norm_ln(e[1], e[2])
                    emit_norm_rest(e[1], e[2])

            def b_pair(hp, qq, kb0):
                """Steps kb0, kb0+1 together: scores issue E,O,E,O (row
                groups alternate so all four streams chain with no PE
                drain bubbles), then both exps, then the 4 lagged PV
                matmuls in one batch - 2 stream-switches per pair instead
                of 6."""
                h_e, h_o = 2 * hp, 2 * hp + 1
                qs = slice(qq * 512, (qq + 1) * 512)
                if kb0 == 0:
                    pvt[(h_e, qq)] = psB.tile([65, 512], f32, tag="pv", bufs=2,
                                              name=f"pv{h_e}{qq}")
                    pvt[(h_o, qq)] = psB.tile([65, 512], f32, tag="pv", bufs=2,
                                              name=f"pv{h_o}{qq}")
                mks, psAs = [], []
                for kb in (kb0, kb0 + 1):
                    mk_t = bpool.tile([128, 512], bf16, tag="mk", bufs=10,
                                      name=f"mk{hp}{qq}_{kb}")
                    nc.sync.dma_start(
                        out=mk_t[:, :],
                        in_=masktd[kb * 128:(kb + 1) * 128, qs])
                    mks.append(mk_t)
                for kb in (kb0, kb0 + 1):
                    psA = psB.tile([128, 1024], f32, tag="sc", bufs=2,
                                   name=f"sA{hp}{qq}_{kb}")
                    kbs = slice(kb * 128, (kb + 1) * 128)
                    nc.tensor.matmul(psA[:, 0:512], kt[hp][0:64, kbs],
                                     qt[hp][0:64, qs], start=True, stop=True)
                    nc.tensor.matmul(psA[:, 512:1024], kt[hp][64:128, kbs],
                                     qt[hp][64:128, qs], start=True, stop=True)
                    psAs.append(psA)
                for i, kb in enumerate((kb0, kb0 + 1)):
                    probs = bpool.tile([128, 1024], bf16, tag="probs", bufs=9,
                                       name=f"pr{hp}{qq}_{kb}")
                    nc.scalar.activation(probs[:, :], psAs[i][:, :], AF.Exp)
                    nc.vector.tensor_tensor(probs[:, 0:512], probs[:, 0:512],
                                            mks[i][:, :], op=MULT)
                    nc.vector.tensor_tensor(probs[:, 512:1024],
                                            probs[:, 512:1024],
                                            mks[i][:, :], op=MULT)
                    pend_pv.append((h_e, qq, kb, probs, 0))
                    pend_pv.append((h_o, qq, kb, probs, 1))
                run_pend_norm()
                pops = []
                while len(pend_pv) > 2 * LAG:
                    pops.append(pend_pv.pop(0))
                # group by head so consecutive PVs accumulate into the same
                # PSUM chain (~170ns issue) instead of alternating
                # accumulators (~300ns re-arm); kb stays ascending per head
                pops.sort(key=lambda e: (e[0], e[2]))
                for ph, pqq, pkb, pprobs, phf in pops:
                    emit_pv(ph, pqq, pkb, pprobs, phf)
                    if pkb == NKB - 1:
                        pend_norm.append([NORM_DELAY, ph, pqq, 0])

            # ---------- pre-work: just enough for segment 0 ----------
            emit_k_half(0, 0, 0)
            emit_k_half(0, 0, 1)
            emit_q_quarter(0, 0)
            for sb in range(3):
                emit_v_chunk(sb)

            # emission schedule: seg index -> {step -> [thunks]}
            def K(db, g, hf):
                return lambda: emit_k_half(db, g, hf)

            def Q(db, qq):
                return lambda: emit_q_quarter(db, qq)

            def V(sb):
                return lambda: emit_v_chunk(sb)

            def O(sb):
                return lambda: emit_out_proj(sb)

            sched = {s: {} for s in range(8)}

            def add(s, step, thunk):
                sched[s].setdefault(step, []).append(thunk)

            # segment 0: remaining K chunks, qt[1] quarter 0, all V chunks
            # (1,0,*) first: it only needs xk-g0 which the sync prefix
            # already delivered; xk-g1..3 stream in on the GpSimd ring,
            # so space the dependent emissions to match arrival
            k_slots = [(0, (1, 0, 0)), (1, (1, 0, 1)), (3, (0, 1, 0)),
                       (4, (0, 1, 1)), (7, (0, 2, 0)), (8, (0, 2, 1)),
                       (11, (0, 3, 0)), (12, (0, 3, 1)), (14, (1, 1, 0)),
                       (15, (1, 1, 1)), (17, (1, 2, 0)), (18, (1, 2, 1)),
                       (20, (1, 3, 0)), (21, (1, 3, 1))]
            for st, (db, g, hf) in k_slots:
                add(0, st, K(db, g, hf))
            add(0, 6, Q(1, 0))
            for kb in range(NKB - 3):
                add(0, kb, V(kb + 3))
            # later q quarters, two segments ahead of use
            add(1, 0, Q(0, 1))
            add(1, 1, Q(1, 1))
            add(3, 0, Q(0, 2))
            add(3, 1, Q(1, 2))
            add(5, 0, Q(0, 3))
            add(5, 1, Q(1, 3))
            # output projection: quarter qq's 4 blocks ride 2 segments later
            for qq in range(3):
                for j in range(4):
                    add(2 * qq + 2, 12 + 2 * j, O(qq * 4 + j))

            for s in range(8):
                qq, hp = divmod(s, 2)
                for kb0 in range(0, NKB, 2):
                    b_pair(hp, qq, kb0)
                    for thunk in (list(sched[s].get(kb0, []))
                                  + list(sched[s].get(kb0 + 1, []))):
                        thunk()

            # ---------- drain ----------
            pend_pv.sort(key=lambda e: (e[0], e[2]))
            while pend_pv:
                ph, pqq, pkb, pprobs, phf = pend_pv.pop(0)
                emit_pv(ph, pqq, pkb, pprobs, phf)
                if pkb == NKB - 1:
                    pend_norm.append([NORM_DELAY, ph, pqq, 0])

            # drain odd heads first: their at2 writeback goes through an
            # SBUF->SBUF DMA, the longest-latency link in the tail chain
            pend_norm.sort(key=lambda e: -(e[1] % 2))
            while pend_norm:
                e = pend_norm.pop(0)
                if e[3] < 1:
                    emit_norm_stg(e[1], e[2], pvt.pop((e[1], e[2])))
                if e[3] < 2:
                    emit_norm_ln(e[1], e[2])
                emit_norm_rest(e[1], e[2])

            for j in range(4):
                emit_out_proj(12 + j)

    _legalize_waits(nc)
    return nc


_program_cache = {}
_last_in_maps = None


def _get_program(with_bias=False):
    key = ("nc", with_bias)
    if key not in _program_cache:
        _program_cache[key] = _build_program(with_bias)
    return _program_cache[key]


def kernel(query, key, value, mask, Wq, bq, Wk, bk, Wv, bv, Wo, bo, **_unused):
    query = np.asarray(query, dtype=np.float32)
    key = np.asarray(key, dtype=np.float32)
    value = np.asarray(value, dtype=np.float32)
    mask = np.asarray(mask)

    with_bias = bool(np.any(np.asarray(bq)) or np.any(np.asarray(bk))
                     or np.any(np.asarray(bv)) or np.any(np.asarray(bo)))

    wqT = np.ascontiguousarray(np.asarray(Wq, np.float32).T).astype(np.float16)
    wkT = np.ascontiguousarray(np.asarray(Wk, np.float32).T).astype(np.float16)
    wvT = np.ascontiguousarray(np.asarray(Wv, np.float32).T).astype(np.float16)
    woT = np.ascontiguousarray(np.asarray(Wo, np.float32).T).astype(np.float16)
    bq_f = np.asarray(bq, np.float32)
    bk_f = np.asarray(bk, np.float32)
    bv_f = np.asarray(bv, np.float32).astype(np.float16)
    bo_f = np.asarray(bo, np.float32).astype(np.float16)

    # bf16 bits for the (0/1) mask: exact; pre-transposed per batch
    mbits = (mask != 0).astype(np.uint16) * np.uint16(0x3F80)
    mbitsT = [np.ascontiguousarray(mbits[b].T) for b in range(B)]

    xT = {}
    for b in range(B):
        xT[("q", b)] = np.ascontiguousarray(query[b].T).astype(np.float16)
        xT[("k", b)] = np.ascontiguousarray(key[b].T).astype(np.float16)
        xT[("v", b)] = np.ascontiguousarray(value[b].T).astype(np.float16)

    in_maps = []
    for c in range(NC):
        b, r = divmod(c, 4)
        qh, hg = divmod(r, 2)
        ds = slice(hg * DG, (hg + 1) * DG)
        qs = slice(qh * QC, (qh + 1) * QC)
        in_maps.append({
            "xqT": np.ascontiguousarray(xT[("q", b)][:, qs]),
            "xkT": xT[("k", b)],
            "xvT": xT[("v", b)],
            "masktd": np.ascontiguousarray(mbitsT[b][:, qs]).view(ml_dtypes.bfloat16),
            "wqT": np.ascontiguousarray(wqT[:, ds]),
            "wkT": np.ascontiguousarray(wkT[:, ds]),
            "wvT": np.ascontiguousarray(wvT[:, ds]),
            "woT": np.ascontiguousarray(woT[ds, :]),
            "bq_d": np.ascontiguousarray(bq_f[ds].reshape(NDB, 128).T),
            "bk_d": np.ascontiguousarray(bk_f[ds].reshape(NDB, 128).T),
            "bv_d": bv_f[ds].reshape(1, DG),
            # apply bo on head-group 0 only so the host sum stays correct
            "bo_d": (bo_f if hg == 0 else np.zeros_like(bo_f)).reshape(1, D),
        })

    global _last_in_maps
    _last_in_maps = in_maps
    nc = _get_program(with_bias)
    res = run_bass_kernel_spmd(nc, in_maps, list(range(NC)))

    out = np.empty((B, S, D), np.float32)
    for b in range(B):
        for qh in range(2):
            c0 = b * 4 + qh * 2
            part = np.asarray(res.results[c0]["out"], np.float32) + \
                np.asarray(res.results[c0 + 1]["out"], np.float32)
            out[b, qh * QC:(qh + 1) * QC, :] = part
    return out


# revision 67
# speedup vs baseline: 1.0187x; 1.0187x over previous
"""Multi-head attention Trainium2 kernel (B=2, S=4096, D=512, H=8).

Sharding: 8 cores = (batch b) x (query half qh: 2048 rows) x
(head group hg: 4 heads = 256 model dims).  Each core:
  - Q projection for its 2048 queries x its 256 dims
  - K/V projections for the full 4096 keys x its 256 dims
  - scores/softmax/PV for its 4 heads x its 2048 queries
  - partial output projection [2048, 512] through its 256 Wo rows
The host sums the two head-groups' partial outputs per (b, qh).

Schedule (v3): 8 segments = (query-quarter qq 0..3) x (head-pair hp 0..1),
16 step-PAIRS each.  Per pair the four score matmuls issue E,O,E,O
(row groups 0:64/64:128 alternate; the PE shares one 2.4GHz column bus,
so batching minimizes full-array stream switches: 2 per pair instead of
6), each step's E|O lands in one merged [128,1024] PSUM tile covered by
a single 1024-wide ScalarE exp, VectorE multiplies the halves by the
shared bf16 mask tile, and the four lagged PV matmuls (va stationary
65-wide: 64 dims + a 2^-64-scaled ones column accumulating the softmax
denominator; contract 128 so PV can never row-tile with anything)
run as one batch.  PSUM: 4 banks scores (bufs=2) + 2 banks PV
accumulators + 2 banks shared projection tag "px".
All projections (K/Q/V chunks, output projection) are interleaved into
the attention steps just ahead of use, so the exp stream starts ~15us
in and the tail is only the last quarter's norm drain.
Softmax normalization (no DRAM bounces): 1/den = exp(-ln(den*2^-64)
- 64*ln2), Ln and Exp sharing one ScalarE activation table; the den
row pre-scale keeps it inside the Ln table's valid window
[3.7e-20, 2e19] for the full f32 range; the [1,512] lnden row is
broadcast to 64 partitions by a tiny f32r PE matmul and the -64*ln2
compensation rides the Exp activation's bias input.
Output projection contracts 128 rows per matmul: at2[hp] stacks each
head-pair's 64+64 dims (odd heads reach partitions 64:128 via an
SBUF->SBUF DMA, the only partition-crossing engine).
DMA: critical prefix (biases, wk, xk-g0) leads the SP ring ahead of
the per-step mask stream; wq/xq lead the GpSimd SWDGE ring followed by
all bulk inputs - keeping long trigger chains off the compute engines
(a trigger chain blocks its engine until the ring drains, which cost
the old layout ~50us of startup).  Output writes go via the Activation
ring, idle after startup.
Known variance: a chip-wide clock governor moves all engines between
~0.65x and ~1.0x in windows; identical code measures ~365-440us.
"""

import numpy as np
import ml_dtypes

import bass_rust
import concourse.bass as bass
import concourse.mybir as mybir
from concourse.bass_utils import run_bass_kernel_spmd
from concourse.tile import TileContext

f32 = mybir.dt.float32
f32r = mybir.dt.float32r
bf16 = mybir.dt.bfloat16
f16 = mybir.dt.float16
AF = mybir.ActivationFunctionType
MULT = mybir.AluOpType.mult

B, S, D, H, HD = 2, 4096, 512, 8, 64
NC = 8
QC = 2048          # query rows per core
DG = 256           # model dims per core (4 heads)
NH = 4             # heads per core
NKB = S // 128     # 32 k-blocks
NDB = DG // 128    # 2 d-blocks (head pairs) per core
NSB = QC // 128    # 16 out s-blocks
NQQ = 4            # query quarters (512 each)

_waitfix = [0]


def _legalize_waits(nc):
    """This walrus build accepts at most one sem-wait per instruction.
    Hoist extra waits onto same-engine NOPs inserted just before."""
    for fn in nc.m.functions:
        for bb in fn.blocks:
            out, changed = [], False
            for inst in bb.instructions:
                si = inst.sync_info
                if si is not None and len(si.on_wait) > 1:
                    waits = list(si.on_wait)
                    for w in waits[:-1]:
                        _waitfix[0] += 1
                        nop = mybir.InstNoOp(
                            name=f"I-waitfix-{_waitfix[0]}", ins=[], outs=[])
                        nop.engine = inst.engine
                        nop.sync_info = bass_rust.SyncInfo(on_wait=[w], on_update=[])
                        out.append(nop)
                    inst.sync_info = bass_rust.SyncInfo(
                        on_wait=[waits[-1]], on_update=list(si.on_update))
                    changed = True
                out.append(inst)
            if changed:
                bb.instructions = out


def _build_program(with_bias=False):
    nc = bass.Bass(target_bir_lowering=False, debug=False)

    xqT = nc.dram_tensor("xqT", [D, QC], f16, kind="ExternalInput")
    xkT = nc.dram_tensor("xkT", [D, S], f16, kind="ExternalInput")
    xvT = nc.dram_tensor("xvT", [D, S], f16, kind="ExternalInput")
    masktd = nc.dram_tensor("masktd", [S, QC], bf16, kind="ExternalInput")
    wqT = nc.dram_tensor("wqT", [D, DG], f16, kind="ExternalInput")
    wkT = nc.dram_tensor("wkT", [D, DG], f16, kind="ExternalInput")
    wvT = nc.dram_tensor("wvT", [D, DG], f16, kind="ExternalInput")
    woT = nc.dram_tensor("woT", [DG, D], f16, kind="ExternalInput")
    bq_d = nc.dram_tensor("bq_d", [128, NDB], f32, kind="ExternalInput")
    bk_d = nc.dram_tensor("bk_d", [128, NDB], f32, kind="ExternalInput")
    bv_d = nc.dram_tensor("bv_d", [1, DG], f16, kind="ExternalInput")
    bo_d = nc.dram_tensor("bo_d", [1, D], f16, kind="ExternalInput")
    outp = nc.dram_tensor("out", [QC, D], f32, kind="ExternalOutput")

    with TileContext(nc) as tc:
        with tc.tile_pool(name="cpool", bufs=1) as cpool, \
             tc.tile_pool(name="bpool", bufs=1) as bpool, \
             tc.tile_pool(name="psB", bufs=1, space="PSUM") as psB:
            # ---- constants ----
            ones_f = cpool.tile([1, 128], f32, tag="ones_f")
            nc.vector.memset(ones_f[:, :], 1.0)
            ones_r = cpool.tile([1, 128], f32r, tag="ones_r")
            nc.vector.tensor_copy(ones_r[:, :], ones_f[:, :])
            negones_f = cpool.tile([65, 128], f32, tag="negones_f")
            nc.vector.memset(negones_f[64:65, :], -1.0)
            negones_r = cpool.tile([65, 128], f32r, tag="negones_r")
            nc.vector.tensor_copy(negones_r[64:65, :], negones_f[64:65, :])
            # den is accumulated pre-scaled by 2^-64 (the va ones column) so
            # it stays inside the Ln table's valid window [3.7e-20, 2.07e19]
            # for the whole f32 range of den; the -64*ln2 term is added back
            # before the exp so invd = exp(-ln(den*2^-64) - 64*ln2) = 1/den.
            ones_h = cpool.tile([1, 128], f16, tag="ones_h")
            nc.vector.tensor_copy(ones_h[:, :], ones_f[:, :])
            nrm_b = cpool.tile([64, 1], f32, tag="nrm_b")
            nc.vector.memset(nrm_b[:, :], -64.0 * float(np.log(2.0)))
            expwarm = cpool.tile([1, 128], f32, tag="expwarm")
            nc.scalar.activation(expwarm[:, :], ones_f[:, :], AF.Exp)

            # ---- critical input prefix on the SP(sync) ring (fast HWDGE,
            # ahead of the mask stream); everything else via GpSimd SWDGE
            # so no compute engine ever blocks issuing DMA triggers ----
            bq_t = cpool.tile([128, NDB], f32, tag="bq")
            bk_t = cpool.tile([128, NDB], f32, tag="bk")
            nc.sync.dma_start(out=bq_t[:, :], in_=bq_d[:, :])
            nc.sync.dma_start(out=bk_t[:, :], in_=bk_d[:, :])
            wk_c, wq_c, wv_c = [], [], []
            for kc in range(4):
                t = cpool.tile([128, DG], f16, tag=f"wk{kc}", name=f"wk{kc}")
                nc.sync.dma_start(out=t[:, :], in_=wkT[kc * 128:(kc + 1) * 128, :])
                wk_c.append(t)
            xk_c = []
            for kc in range(4):
                xk_c.append(cpool.tile([128, S], f16, tag=f"xk{kc}", name=f"xk{kc}"))
            for kc in range(4):
                nc.sync.dma_start(out=xk_c[kc][:, 0:1024],
                                  in_=xkT[kc * 128:(kc + 1) * 128, 0:1024])
            # wq + xq-q0 go at the FRONT of the GpSimd ring so they stream
            # in parallel with the sync-ring prefix
            for kc in range(4):
                t = cpool.tile([128, DG], f16, tag=f"wq{kc}", name=f"wq{kc}")
                nc.gpsimd.dma_start(out=t[:, :], in_=wqT[kc * 128:(kc + 1) * 128, :])
                wq_c.append(t)
            xq_c = []
            for kc in range(4):
                xq_c.append(cpool.tile([128, QC], f16, tag=f"xq{kc}", name=f"xq{kc}"))
            for kc in range(4):
                nc.gpsimd.dma_start(out=xq_c[kc][:, 0:512],
                                    in_=xqT[kc * 128:(kc + 1) * 128, 0:512])
            bv_t = cpool.tile([1, DG], f16, tag="bv")
            bo_t = cpool.tile([1, D], f16, tag="bo")

            # ---- bulk inputs on GpSimd SWDGE, in dependency-priority order
            xv_c = []
            for kc in range(4):
                xv_c.append(cpool.tile([128, S], f16, tag=f"xv{kc}", name=f"xvc{kc}"))
            for kc in range(4):
                nc.gpsimd.dma_start(out=xv_c[kc][:, 0:1024],
                                    in_=xvT[kc * 128:(kc + 1) * 128, 0:1024])
            for kc in range(4):
                t = cpool.tile([128, DG], f16, tag=f"wv{kc}", name=f"wv{kc}")
                nc.gpsimd.dma_start(out=t[:, :], in_=wvT[kc * 128:(kc + 1) * 128, :])
                wv_c.append(t)
            nc.gpsimd.dma_start(out=bv_t[:, :], in_=bv_d[:, :])
            for kc in range(4):
                nc.gpsimd.dma_start(
                    out=xk_c[kc][:, 1024:2048], in_=xkT[kc * 128:(kc + 1) * 128, 1024:2048])
            for kc in range(4):
                nc.gpsimd.dma_start(out=xv_c[kc][:, 1024:2048],
                                    in_=xvT[kc * 128:(kc + 1) * 128, 1024:2048])
            for kc in range(4):
                nc.gpsimd.dma_start(
                    out=xk_c[kc][:, 2048:3072], in_=xkT[kc * 128:(kc + 1) * 128, 2048:3072])
            for kc in range(4):
                nc.gpsimd.dma_start(
                    out=xk_c[kc][:, 3072:4096], in_=xkT[kc * 128:(kc + 1) * 128, 3072:4096])
            for kc in range(4):
                nc.gpsimd.dma_start(out=xv_c[kc][:, 2048:3072],
                                    in_=xvT[kc * 128:(kc + 1) * 128, 2048:3072])
            wo2 = []
            for hp in range(NDB):
                t = cpool.tile([128, D], f16, tag=f"wo2{hp}", name=f"wo2{hp}")
                nc.gpsimd.dma_start(out=t[:, :], in_=woT[hp * 128:(hp + 1) * 128, :])
                wo2.append(t)
            nc.gpsimd.dma_start(out=bo_t[:, :], in_=bo_d[:, :])
            for kc in range(4):
                nc.gpsimd.dma_start(out=xv_c[kc][:, 3072:4096],
                                    in_=xvT[kc * 128:(kc + 1) * 128, 3072:4096])
            for qq in range(1, 4):
                for kc in range(4):
                    nc.gpsimd.dma_start(
                        out=xq_c[kc][:, qq * 512:(qq + 1) * 512],
                        in_=xqT[kc * 128:(kc + 1) * 128, qq * 512:(qq + 1) * 512])

            # ---- PE warmup (ramp cover while DMA streams) ----
            ones_w = cpool.tile([128, 512], f16, tag="ones_w")
            nc.vector.memset(ones_w[:, :], 1.0)
            wps = psB.tile([128, 1024], f32, tag="sc", bufs=2, name="wps")
            for r in range(22):
                nc.tensor.matmul(wps[:, 0:512], ones_w[:, 0:128],
                                 ones_w[:, :], start=(r == 0), stop=(r == 21))
            wout = cpool.tile([1, 512], f32, tag="wout")
            nc.vector.tensor_copy(wout[:, :], wps[0:1, 0:512])

            # ---- persistent activations ----
            qt = [cpool.tile([128, QC], f16, tag=f"qt{db}", name=f"qt{db}")
                  for db in range(NDB)]
            kt = [cpool.tile([128, S], f16, tag=f"kt{db}", name=f"kt{db}")
                  for db in range(NDB)]
            va = [cpool.tile([128, NH * 65], bf16, tag=f"va{sb}", name=f"va{sb}")
                  for sb in range(NKB)]
            # at2[hp]: head 2hp dims at partitions 0:64, head 2hp+1 at 64:128
            at2 = [cpool.tile([128, QC], f16, tag=f"at2{hp}", name=f"at2{hp}")
                   for hp in range(NDB)]

            # ---------- projection emitters (all via 1-bank "px" tag) ----------
            def emit_k_half(db, g, hf):
                """kt[db][:, g*1024+hf*512 : ...+512] from xk chunk."""
                ps = psB.tile([128, 512], f32, tag="px", bufs=2,
                              name=f"kps{db}{g}{hf}")
                cs = slice(g * 1024 + hf * 512, g * 1024 + (hf + 1) * 512)
                for kc in range(4):
                    nc.tensor.matmul(ps[:, :],
                                     wk_c[kc][:, db * 128:(db + 1) * 128],
                                     xk_c[kc][:, cs],
                                     start=(kc == 0), stop=(kc == 3))
                if with_bias:
                    nc.scalar.activation(kt[db][:, cs], ps[:, :],
                                         AF.Identity, bias=bk_t[:, db:db + 1])
                else:
                    with nc.allow_low_precision(reason="f16 kt"):
                        nc.vector.tensor_copy(kt[db][:, cs], ps[:, :])

            def emit_q_quarter(db, qq):
                """qt[db][:, qq*512:(qq+1)*512]."""
                ps = psB.tile([128, 512], f32, tag="px", bufs=2,
                              name=f"qps{db}{qq}")
                cs = slice(qq * 512, (qq + 1) * 512)
                for kc in range(4):
                    nc.tensor.matmul(ps[:, :],
                                     wq_c[kc][:, db * 128:(db + 1) * 128],
                                     xq_c[kc][:, cs],
                                     start=(kc == 0), stop=(kc == 3))
                if with_bias:
                    nc.scalar.activation(qt[db][:, cs], ps[:, :],
                                         AF.Identity, bias=bq_t[:, db:db + 1])
                else:
                    with nc.allow_low_precision(reason="f16 qt"):
                        nc.vector.tensor_copy(qt[db][:, cs], ps[:, :])

            def emit_v_chunk(sb):
                ps = psB.tile([128, 512], f32, tag="px", bufs=2,
                              name=f"vps{sb}")[:, 0:DG]
                for kc in range(4):
                    nc.tensor.matmul(ps[:, :],
                                     xv_c[kc][:, sb * 128:(sb + 1) * 128],
                                     wv_c[kc][:, :],
                                     start=(kc == 0),
                                     stop=(kc == 3 and not with_bias))
                if with_bias:
                    nc.tensor.matmul(ps[:, :], ones_h[0:1, :], bv_t[0:1, :],
                                     start=False, stop=True)
                dst = va[sb][:, :].rearrange("p (h c) -> p h c", c=65)
                src = ps[:, :].rearrange("p (h c) -> p h c", c=64)
                with nc.allow_low_precision(reason="bf16 va"):
                    nc.vector.tensor_copy(dst[:, :, 0:64], src[:, :, :])
                nc.vector.memset(dst[:, :, 64:65], 2.0 ** -64)

            def emit_out_proj(sb):
                po = psB.tile([128, 512], f32, tag="px", bufs=2,
                              name=f"po{sb}")
                for hp in range(NDB):
                    nc.tensor.matmul(
                        po[:, :],
                        at2[hp][:, sb * 128:(sb + 1) * 128],
                        wo2[hp][:, :],
                        start=(hp == 0), stop=(hp == NDB - 1 and not with_bias))
                if with_bias:
                    nc.tensor.matmul(po[:, :], ones_h[0:1, :], bo_t[0:1, :],
                                     start=False, stop=True)
                osb = bpool.tile([128, D], f32, tag="osb", bufs=2,
                                 name=f"osb{sb}")
                nc.vector.tensor_copy(osb[:, :], po[:, :])
                nc.scalar.dma_start(out=outp[sb * 128:(sb + 1) * 128, :],
                                    in_=osb[:, :])

            # ---------- normalization: 1/den = exp(-ln(den)), all on
            # ScalarE (Ln and Exp share one activation table) ----------
            stgs, lndens, stg2s = {}, {}, {}

            def emit_norm_stg(h, qq, pv):
                stg = bpool.tile([65, 512], f32, tag="stg", bufs=3,
                                 name=f"stg{h}{qq}")
                nc.vector.tensor_copy(stg[:, :], pv[:, :])
                stgs[(h, qq)] = stg

            def emit_norm_ln(h, qq):
                stg = stgs[(h, qq)]
                lnden = bpool.tile([65, 512], f32r, tag="lnden", bufs=3,
                                   name=f"ln{h}{qq}")
                nc.scalar.activation(lnden[64:65, :], stg[64:65, :], AF.Ln)
                lndens[(h, qq)] = lnden

            def emit_norm_rest(h, qq):
                stg = stgs.pop((h, qq))
                lnden = lndens.pop((h, qq))
                qs = slice(qq * 512, (qq + 1) * 512)
                bc = psB.tile([128, 512], f32, tag="px", bufs=2,
                              name=f"bc{h}{qq}")
                nc.tensor.matmul(bc[0:64, :], negones_r[64:65, 0:64],
                                 lnden[64:65, :], start=True, stop=True)
                # invd = exp(-lnden - 64*ln2) = 1/den (the 2^-64 den
                # pre-scale is undone by the activation bias)
                invd = bpool.tile([64, 512], f32, tag="invd", bufs=2,
                                  name=f"invd{h}{qq}")
                nc.scalar.activation(invd[:, :], bc[0:64, :], AF.Exp,
                                     bias=nrm_b[:, 0:1])
                with nc.allow_low_precision(reason="f16 at"):
                    if h % 2 == 0:
                        nc.vector.tensor_tensor(
                            at2[h // 2][0:64, qs],
                            stg[0:64, :], invd[:, :], op=MULT)
                    else:
                        # odd heads land at partitions 64:128 of at2; only
                        # DMA crosses partitions, so stage through SBUF
                        ato = bpool.tile([64, 512], f16, tag="ato", bufs=2,
                                         name=f"ato{h}{qq}")
                        nc.vector.tensor_tensor(
                            ato[:, :], stg[0:64, :], invd[:, :], op=MULT)
                        nc.gpsimd.dma_start(
                            out=at2[h // 2][64:128, qs], in_=ato[:, :])

            # ---------- attention ----------
            LAG = 3            # probs-entries PV trails behind scores
            NORM_DELAY = 4     # in step-PAIRS (run_pend_norm per pair)
            pvt = {}
            pend_pv = []       # (h, qq, kb, probs, half)
            pend_norm = []     # [delay, h, qq, stage]

            def emit_pv(h, qq, kb, probs, hf):
                nc.tensor.matmul(
                    pvt[(h, qq)][:, :],
                    va[kb][:, h * 65:(h + 1) * 65],
                    probs[:, hf * 512:(hf + 1) * 512],
                    start=(kb == 0), stop=(kb == NKB - 1))

            def run_pend_norm():
                for ent in pend_norm:
                    ent[0] -= 1
                    if ent[0] == NORM_DELAY - 1 and ent[3] == 0:
                        emit_norm_stg(ent[1], ent[2], pvt.pop((ent[1], ent[2])))
                        ent[3] = 1
                    elif ent[0] == NORM_DELAY - 3 and ent[3] == 1:
                        emit_norm_ln(ent[1], ent[2])
                        ent[3] = 2
                while pend_norm and pend_norm[0][0] <= 0:
                    e = pend_norm.pop(0)
                    if e[3] < 1:
                        emit_norm_stg(e[1], e[2], pvt.pop((e[1], e[2])))
                    if e[3] < 2:
                        emit_norm_ln(e[1], e[2])
                    emit_norm_rest(e[1], e[2])

            def b_pair(hp, qq, kb0):
                """Steps kb0, kb0+1 together: scores issue E,O,E,O (row
                groups alternate so all four streams chain with no PE
                drain bubbles), then both exps, then the 4 lagged PV
                matmuls in one batch - 2 stream-switches per pair instead
                of 6."""
                h_e, h_o = 2 * hp, 2 * hp + 1
                qs = slice(qq * 512, (qq + 1) * 512)
                if kb0 == 0:
                    pvt[(h_e, qq)] = psB.tile([65, 512], f32, tag="pv", bufs=2,
                                              name=f"pv{h_e}{qq}")
                    pvt[(h_o, qq)] = psB.tile([65, 512], f32, tag="pv", bufs=2,
                                              name=f"pv{h_o}{qq}")
                mks, psAs = [], []
                for kb in (kb0, kb0 + 1):
                    mk_t = bpool.tile([128, 512], bf16, tag="mk", bufs=10,
                                      name=f"mk{hp}{qq}_{kb}")
                    nc.sync.dma_start(
                        out=mk_t[:, :],
                        in_=masktd[kb * 128:(kb + 1) * 128, qs])
                    mks.append(mk_t)
                for kb in (kb0, kb0 + 1):
                    psA = psB.tile([128, 1024], f32, tag="sc", bufs=2,
                                   name=f"sA{hp}{qq}_{kb}")
                    kbs = slice(kb * 128, (kb + 1) * 128)
                    nc.tensor.matmul(psA[:, 0:512], kt[hp][0:64, kbs],
                                     qt[hp][0:64, qs], start=True, stop=True)
                    nc.tensor.matmul(psA[:, 512:1024], kt[hp][64:128, kbs],
                                     qt[hp][64:128, qs], start=True, stop=True)
                    psAs.append(psA)
                for i, kb in enumerate((kb0, kb0 + 1)):
                    probs = bpool.tile([128, 1024], bf16, tag="probs", bufs=9,
                                       name=f"pr{hp}{qq}_{kb}")
                    nc.scalar.activation(probs[:, :], psAs[i][:, :], AF.Exp)
                    nc.vector.tensor_tensor(probs[:, 0:512], probs[:, 0:512],
                                            mks[i][:, :], op=MULT)
                    nc.vector.tensor_tensor(probs[:, 512:1024],
                                            probs[:, 512:1024],
                                            mks[i][:, :], op=MULT)
                    pend_pv.append((h_e, qq, kb, probs, 0))
                    pend_pv.append((h_o, qq, kb, probs, 1))
                run_pend_norm()
                pops = []
                while len(pend_pv) > 2 * LAG:
                    pops.append(pend_pv.pop(0))
                # group by head so consecutive PVs accumulate into the same
                # PSUM chain (~170ns issue) instead of alternating
                # accumulators (~300ns re-arm); kb stays ascending per head
                pops.sort(key=lambda e: (e[0], e[2]))
                for ph, pqq, pkb, pprobs, phf in pops:
                    emit_pv(ph, pqq, pkb, pprobs, phf)
                    if pkb == NKB - 1:
                        pend_norm.append([NORM_DELAY, ph, pqq, 0])

            # ---------- pre-work: just enough for segment 0 ----------
            emit_k_half(0, 0, 0)
            emit_k_half(0, 0, 1)
            emit_q_quarter(0, 0)
            for sb in range(3):
                emit_v_chunk(sb)

            # emission schedule: seg index -> {step -> [thunks]}
            def K(db, g, hf):
                return lambda: emit_k_half(db, g, hf)

            def Q(db, qq):
                return lambda: emit_q_quarter(db, qq)

            def V(sb):
                return lambda: emit_v_chunk(sb)

            def O(sb):
                return lambda: emit_out_proj(sb)

            sched = {s: {} for s in range(8)}

            def add(s, step, thunk):
                sched[s].setdefault(step, []).append(thunk)

            # segment 0: remaining K chunks, qt[1] quarter 0, all V chunks
            # (1,0,*) first: it only needs xk-g0 which the sync prefix
            # already delivered; xk-g1..3 stream in on the GpSimd ring,
            # so space the dependent emissions to match arrival
            k_slots = [(0, (1, 0, 0)), (1, (1, 0, 1)), (3, (0, 1, 0)),
                       (4, (0, 1, 1)), (7, (0, 2, 0)), (8, (0, 2, 1)),
                       (11, (0, 3, 0)), (12, (0, 3, 1)), (14, (1, 1, 0)),
                       (15, (1, 1, 1)), (17, (1, 2, 0)), (18, (1, 2, 1)),
                       (20, (1, 3, 0)), (21, (1, 3, 1))]
            for st, (db, g, hf) in k_slots:
                add(0, st, K(db, g, hf))
            add(0, 6, Q(1, 0))
            for kb in range(NKB - 3):
                add(0, kb, V(kb + 3))
            # later q quarters, two segments ahead of use
            add(1, 0, Q(0, 1))
            add(1, 1, Q(1, 1))
            add(3, 0, Q(0, 2))
            add(3, 1, Q(1, 2))
            add(5, 0, Q(0, 3))
            add(5, 1, Q(1, 3))
            # output projection: quarter qq's 4 blocks ride 2 segments later
            for qq in range(3):
                for j in range(4):
                    add(2 * qq + 2, 12 + 2 * j, O(qq * 4 + j))

            for s in range(8):
                qq, hp = divmod(s, 2)
                for kb0 in range(0, NKB, 2):
                    b_pair(hp, qq, kb0)
                    for thunk in (list(sched[s].get(kb0, []))
                                  + list(sched[s].get(kb0 + 1, []))):
                        thunk()

            # ---------- drain ----------
            pend_pv.sort(key=lambda e: (e[0], e[2]))
            while pend_pv:
                ph, pqq, pkb, pprobs, phf = pend_pv.pop(0)
                emit_pv(ph, pqq, pkb, pprobs, phf)
                if pkb == NKB - 1:
                    pend_norm.append([NORM_DELAY, ph, pqq, 0])

            # drain odd heads first: their at2 writeback goes through an
            # SBUF->SBUF DMA, the longest-latency link in the tail chain
            pend_norm.sort(key=lambda e: -(e[1] % 2))
            while pend_norm:
                e = pend_norm.pop(0)
                if e[3] < 1:
                    emit_norm_stg(e[1], e[2], pvt.pop((e[1], e[2])))
                if e[3] < 2:
                    emit_norm_ln(e[1], e[2])
                emit_norm_rest(e[1], e[2])

            for j in range(4):
                emit_out_proj(12 + j)

    _legalize_waits(nc)
    return nc


_program_cache = {}
_last_in_maps = None


def _get_program(with_bias=False):
    key = ("nc", with_bias)
    if key not in _program_cache:
        _program_cache[key] = _build_program(with_bias)
    return _program_cache[key]


def kernel(query, key, value, mask, Wq, bq, Wk, bk, Wv, bv, Wo, bo, **_unused):
    query = np.asarray(query, dtype=np.float32)
    key = np.asarray(key, dtype=np.float32)
    value = np.asarray(value, dtype=np.float32)
    mask = np.asarray(mask)

    with_bias = bool(np.any(np.asarray(bq)) or np.any(np.asarray(bk))
                     or np.any(np.asarray(bv)) or np.any(np.asarray(bo)))

    wqT = np.ascontiguousarray(np.asarray(Wq, np.float32).T).astype(np.float16)
    wkT = np.ascontiguousarray(np.asarray(Wk, np.float32).T).astype(np.float16)
    wvT = np.ascontiguousarray(np.asarray(Wv, np.float32).T).astype(np.float16)
    woT = np.ascontiguousarray(np.asarray(Wo, np.float32).T).astype(np.float16)
    bq_f = np.asarray(bq, np.float32)
    bk_f = np.asarray(bk, np.float32)
    bv_f = np.asarray(bv, np.float32).astype(np.float16)
    bo_f = np.asarray(bo, np.float32).astype(np.float16)

    # bf16 bits for the (0/1) mask: exact; pre-transposed per batch
    mbits = (mask != 0).astype(np.uint16) * np.uint16(0x3F80)
    mbitsT = [np.ascontiguousarray(mbits[b].T) for b in range(B)]

    xT = {}
    for b in range(B):
        xT[("q", b)] = np.ascontiguousarray(query[b].T).astype(np.float16)
        xT[("k", b)] = np.ascontiguousarray(key[b].T).astype(np.float16)
        xT[("v", b)] = np.ascontiguousarray(value[b].T).astype(np.float16)

    in_maps = []
    for c in range(NC):
        b, r = divmod(c, 4)
        qh, hg = divmod(r, 2)
        ds = slice(hg * DG, (hg + 1) * DG)
        qs = slice(qh * QC, (qh + 1) * QC)
        in_maps.append({
            "xqT": np.ascontiguousarray(xT[("q", b)][:, qs]),
            "xkT": xT[("k", b)],
            "xvT": xT[("v", b)],
            "masktd": np.ascontiguousarray(mbitsT[b][:, qs]).view(ml_dtypes.bfloat16),
            "wqT": np.ascontiguousarray(wqT[:, ds]),
            "wkT": np.ascontiguousarray(wkT[:, ds]),
            "wvT": np.ascontiguousarray(wvT[:, ds]),
            "woT": np.ascontiguousarray(woT[ds, :]),
            "bq_d": np.ascontiguousarray(bq_f[ds].reshape(NDB, 128).T),
            "bk_d": np.ascontiguousarray(bk_f[ds].reshape(NDB, 128).T),
            "bv_d": bv_f[ds].reshape(1, DG),
            # apply bo on head-group 0 only so the host sum stays correct
            "bo_d": (bo_f if hg == 0 else np.zeros_like(bo_f)).reshape(1, D),
        })

    global _last_in_maps
    _last_in_maps = in_maps
    nc = _get_program(with_bias)
    res = run_bass_kernel_spmd(nc, in_maps, list(range(NC)))

    out = np.empty((B, S, D), np.float32)
    for b in range(B):
        for qh in range(2):
            c0 = b * 4 + qh * 2
            part = np.asarray(res.results[c0]["out"], np.float32) + \
                np.asarray(res.results[c0 + 1]["out"], np.float32)
            out[b, qh * QC:(qh + 1) * QC, :] = part
    return out
